# revision 20
# baseline (speedup 1.0000x reference)
"""VQ codebook kernel (nn_ApplyKmeans): dist = ||x||^2 - 2 x@C + Cnorm; argmin; gather.

Strategy (8 NeuronCores, data-parallel over rows of x):
  - Host: shard x by rows, cast to fp8 e4m3, and pre-tile into the DoubleRow
    SBUF layout: contraction dim d = c*256 + ko*128 + ki over CH=3 chunks,
    so each matmul contracts 256 dims (2 fp8 weights per PE cell).
  - Device per core (PE-bound design): raw scores = x.T @ C in fp8 e4m3 with
    perf_mode=DoubleRow (6 N=512 matmuls per 128-row subtile, ~216 ns each,
    ~2x the fp16 roofline), ScalarE+VectorE copy PSUM -> SBUF fp16 (one half
    each), DMA exports the raw score matrix (64 MB/core).
  - Host: subtract Cnorm/2 in fp32, take the fp8-score argmax, and exactly
    re-score every candidate within CAP_THETA of the per-row max (~4/row;
    fp8 score noise is std ~1.04, observed absmax ~5.3, so theta=13 captures
    the true argmax with enormous margin). A final float64 full-row re-score
    covers rows whose exact top-2 margin is below fp32 rescore noise.
"""

import sys

sys.path.insert(0, "/opt/trn_rl_repo")

import numpy as np
import ml_dtypes

import concourse.bass as bass
import concourse.mybir as mybir
from concourse import bacc
from concourse.tile import TileContext
from concourse.bass_utils import run_bass_kernel_spmd

N, D, K = 262144, 768, 1024
NCORES = 8
NSH = N // NCORES            # 32768 rows per core
CH = 3                       # DoubleRow contraction chunks of 256
MT = 512                     # rows per DMA tile
NOT = NSH // MT              # 64 outer tiles
CAP_THETA = 13.0             # fp8-score candidate-capture margin
FINE_THETA = 1e-3            # fp32-rescore tie margin -> float64 recheck

E4 = ml_dtypes.float8_e4m3


def build_kernel():
    nc = bacc.Bacc()
    xt_ext = nc.declare_dram_parameter("xt", [NOT, 128, CH, 2, MT], mybir.dt.float8e4, isOutput=False)
    cb_ext = nc.declare_dram_parameter("cb", [128, CH, 2, K], mybir.dt.float8e4, isOutput=False)
    sc_ext = nc.declare_dram_parameter("sc", [NOT, 128, MT // 128, K], mybir.dt.float16, isOutput=True)

    with TileContext(nc) as tc:
        with (
            tc.tile_pool(name="const", bufs=1) as const_pool,
            tc.tile_pool(name="xp", bufs=3) as xpool,
            tc.tile_pool(name="scp", bufs=4) as scpool,
            tc.tile_pool(name="ps", bufs=4, space="PSUM") as pspool,
        ):
            # Codebook on the Sync HWDGE ring, first x tile per-chunk on the
            # ScalarE ring: both chunk-0s land ~2us after the preamble so the
            # matmul stream starts as early as possible.
            csb = const_pool.tile([128, CH, 2, K], mybir.dt.float8e4)
            for c in range(CH):
                nc.sync.dma_start(out=csb[:, c, :, :], in_=cb_ext[:, c, :, :])


            for ot in range(NOT):
                xtile = xpool.tile([128, CH, 2, MT], mybir.dt.float8e4, tag="xt")
                if ot == 0:
                    for c in range(CH):
                        nc.scalar.dma_start(out=xtile[:, c, :, :], in_=xt_ext[0, :, c, :, :])
                else:
                    nc.sync.dma_start(out=xtile[:], in_=xt_ext[ot])
                ssc = scpool.tile([128, MT // 128, K], mybir.dt.float16, tag="sc")
                for j in range(MT // 128):
                    psum = pspool.tile([128, K], mybir.dt.float32, space="PSUM", tag="ps")
                    for h in range(2):
                        for c in range(CH):
                            nc.tensor.matmul(
                                out=psum[:, h * 512:(h + 1) * 512],
                                lhsT=xtile[:, c, :, j * 128:(j + 1) * 128],
                                rhs=csb[:, c, :, h * 512:(h + 1) * 512],
                                start=(c == 0),
                                stop=(c == CH - 1),
                                perf_mode=mybir.MatmulPerfMode.DoubleRow,
                            )
                    # PSUM fp32 -> SBUF fp16, one half per engine; h-outer MM
                    # order lets the ScalarE half start while half 1 computes
                    nc.scalar.copy(out=ssc[:, j, 0:512], in_=psum[:, 0:512])
                    nc.vector.tensor_copy(out=ssc[:, j, 512:1024], in_=psum[:, 512:1024])
                    if ot == NOT - 1:
                        # final tile: per-subtile stores on the (now idle) Sync
                        # ring so the tail doesn't wait on the SWDGE backlog
                        nc.sync.dma_start(out=sc_ext[ot, :, j, :], in_=ssc[:, j, :])
                if ot < NOT - 1:
                    if ot >= NOT - 8:
                        # late tiles: Sync HWDGE ring (x prefetch is nearly
                        # done) so the teardown never waits on SWDGE drain
                        nc.sync.dma_start(out=sc_ext[ot], in_=ssc[:])
                    else:
                        # score export on the GpSimd SWDGE ring: keeps the
                        # Sync HWDGE ring free for x prefetch
                        nc.gpsimd.dma_start(out=sc_ext[ot], in_=ssc[:])

    nc.finalize()
    return nc


def _prep_core(args):
    x, c = args
    x8 = x[c * NSH:(c + 1) * NSH].astype(E4)
    # xt[ot, ki, ch, ko, j*128+q] = x8[512*ot + 4*q + j, ch*256 + ko*128 + ki]
    v = x8.reshape(NOT, 128, 4, CH, 2, 128)      # [ot, q, j, ch, ko, ki]
    v = v.transpose(0, 5, 3, 4, 2, 1)            # [ot, ki, ch, ko, j, q]
    return np.ascontiguousarray(v).reshape(NOT, 128, CH, 2, MT)


def prepare_in_maps(x, C, Cnorm):
    x = np.ascontiguousarray(np.asarray(x, dtype=np.float32))
    C = np.ascontiguousarray(np.asarray(C, dtype=np.float32))

    from concurrent.futures import ThreadPoolExecutor
    with ThreadPoolExecutor(max_workers=8) as ex:
        xts = list(ex.map(_prep_core, [(x, c) for c in range(NCORES)]))

    # cb[ki, ch, ko, k] = C[ch*256 + ko*128 + ki, k]
    cb = np.ascontiguousarray(C.astype(E4).reshape(CH, 2, 128, K).transpose(2, 0, 1, 3))
    return [{"xt": xts[c], "cb": cb} for c in range(NCORES)]


def postprocess(results, x, C, Cnorm):
    """Capture candidates from fp8 scores, exactly re-score, gather."""
    x = np.asarray(x, dtype=np.float32)
    C = np.asarray(C, dtype=np.float32)
    bias = np.asarray(Cnorm, dtype=np.float32).reshape(K) * 0.5
    CT = np.ascontiguousarray(C.T)

    all_rows = []
    all_ks = []
    for c in range(NCORES):
        # sc[ot, p, j, k] holds the fp8 raw score of shard row 512*ot + 4*p + j
        sf = results[c]["sc"].reshape(NSH, K).astype(np.float32)
        sf -= bias
        m1 = sf.max(axis=1)
        rows_c, ks_c = np.nonzero(sf > (m1 - CAP_THETA)[:, None])
        all_rows.append(rows_c.astype(np.int64) + c * NSH)
        all_ks.append(ks_c.astype(np.int32))
    rows = np.concatenate(all_rows)     # sorted ascending by construction
    ks = np.concatenate(all_ks)

    # exact fp32 re-score of every candidate pair, grouped by codeword so
    # each group is one BLAS matvec (avoids a 1.5 GB codebook gather)
    order = np.argsort(ks, kind="stable")
    rs = rows[order]
    kss = ks[order]
    s_srt = np.empty(len(rows), dtype=np.float32)
    bounds = np.searchsorted(kss, np.arange(K + 1))
    for k in range(K):
        a, b = bounds[k], bounds[k + 1]
        if a != b:
            s_srt[a:b] = x[rs[a:b]] @ CT[k] - bias[k]
    s_ex = np.empty(len(rows), dtype=np.float32)
    s_ex[order] = s_srt

    # segmented argmax over candidate pairs (rows are sorted ascending)
    starts = np.flatnonzero(np.r_[True, np.diff(rows) != 0])
    counts = np.diff(np.r_[starts, len(rows)])
    seg_rows = rows[starts]
    seg_max = np.maximum.reduceat(s_ex, starts)
    win = s_ex == seg_max.repeat(counts)
    idx = np.empty(N, dtype=np.int64)
    idx[rows[win]] = ks[win]           # every row has >= 1 candidate (its own max)

    # float64 full-row recheck where the exact top-2 margin is inside fp32
    # noise (or the fp32 re-score produced an exact tie)
    s2 = s_ex.copy()
    s2[win] = -np.inf
    seg_second = np.maximum.reduceat(s2, starts)
    nwin = np.add.reduceat(win.astype(np.int64), starts)
    margin = np.where(nwin > 1, 0.0, seg_max - seg_second)
    tie_rows = seg_rows[margin < FINE_THETA]
    if tie_rows.size:
        xr = x[tie_rows].astype(np.float64)
        Cnorm64 = np.asarray(Cnorm, dtype=np.float64).reshape(1, K)
        dist = (
            np.sum(xr * xr, axis=1, keepdims=True)
            - 2.0 * (xr @ C.astype(np.float64))
            + Cnorm64
        )
        idx[tie_rows] = np.argmin(dist, axis=1)

    return CT[idx]


def kernel(x, C, Cnorm):
    in_maps = prepare_in_maps(x, C, Cnorm)
    nc = build_kernel()
    res = run_bass_kernel_spmd(nc, in_maps, core_ids=list(range(NCORES))).results
    return postprocess(res, x, C, Cnorm)


# revision 26
# speedup vs baseline: 1.0125x; 1.0125x over previous
"""VQ codebook kernel (nn_ApplyKmeans): dist = ||x||^2 - 2 x@C + Cnorm; argmin; gather.

Strategy (8 NeuronCores, data-parallel over rows of x):
  - Host: shard x by rows, cast to fp8 e4m3, and pre-tile into the DoubleRow
    SBUF layout: contraction dim d = c*256 + ko*128 + ki over CH=3 chunks,
    so each matmul contracts 256 dims (2 fp8 weights per PE cell).
  - Device per core (PE-bound design): raw scores = x.T @ C in fp8 e4m3 with
    perf_mode=DoubleRow (6 N=512 matmuls per 128-row subtile, ~216 ns each,
    ~2x the fp16 roofline), ScalarE+VectorE copy PSUM -> SBUF fp16 (one half
    each), DMA exports the raw score matrix (64 MB/core).
  - Host: subtract Cnorm/2 in fp32, take the fp8-score argmax, and exactly
    re-score every candidate within CAP_THETA of the per-row max (~4/row;
    fp8 score noise is std ~1.04, observed absmax ~5.3, so theta=13 captures
    the true argmax with enormous margin). A final float64 full-row re-score
    covers rows whose exact top-2 margin is below fp32 rescore noise.
"""

import sys

sys.path.insert(0, "/opt/trn_rl_repo")

import numpy as np
import ml_dtypes

import concourse.bass as bass
import concourse.mybir as mybir
from concourse import bacc
from concourse.tile import TileContext
from concourse.bass_utils import run_bass_kernel_spmd

N, D, K = 262144, 768, 1024
NCORES = 8
NSH = N // NCORES            # 32768 rows per core
CH = 3                       # DoubleRow contraction chunks of 256
MT = 512                     # rows per DMA tile
NOT = NSH // MT              # 64 outer tiles
CAP_THETA = 13.0             # fp8-score candidate-capture margin
FINE_THETA = 1e-3            # fp32-rescore tie margin -> float64 recheck

E4 = ml_dtypes.float8_e4m3


def build_kernel():
    nc = bacc.Bacc()
    xt_ext = nc.declare_dram_parameter("xt", [NOT, 128, CH, 2, MT], mybir.dt.float8e4, isOutput=False)
    cb_ext = nc.declare_dram_parameter("cb", [128, CH, 2, K], mybir.dt.float8e4, isOutput=False)
    sc_ext = nc.declare_dram_parameter("sc", [NOT, 128, MT // 128, K], mybir.dt.float16, isOutput=True)

    with TileContext(nc) as tc:
        with (
            tc.tile_pool(name="const", bufs=1) as const_pool,
            tc.tile_pool(name="xp", bufs=3) as xpool,
            tc.tile_pool(name="scp", bufs=4) as scpool,
            tc.tile_pool(name="ps", bufs=4, space="PSUM") as pspool,
        ):
            # Codebook on the Sync HWDGE ring, first x tile per-chunk on the
            # ScalarE ring: both chunk-0s land ~2us after the preamble so the
            # matmul stream starts as early as possible.
            csb = const_pool.tile([128, CH, 2, K], mybir.dt.float8e4)
            for c in range(CH):
                nc.sync.dma_start(out=csb[:, c, :, :], in_=cb_ext[:, c, :, :])



            for ot in range(NOT):
                xtile = xpool.tile([128, CH, 2, MT], mybir.dt.float8e4, tag="xt")
                if ot == 0:
                    for c in range(CH):
                        nc.scalar.dma_start(out=xtile[:, c, :, :], in_=xt_ext[0, :, c, :, :])
                else:
                    nc.sync.dma_start(out=xtile[:], in_=xt_ext[ot])
                if ot % 2 == 0:
                    ssc2 = scpool.tile([128, 2, MT // 128, K], mybir.dt.float16, tag="sc")
                ssc = ssc2[:, ot % 2]
                for j in range(MT // 128):
                    psum = pspool.tile([128, K], mybir.dt.float32, space="PSUM", tag="ps")
                    for h in range(2):
                        for c in range(CH):
                            nc.tensor.matmul(
                                out=psum[:, h * 512:(h + 1) * 512],
                                lhsT=xtile[:, c, :, j * 128:(j + 1) * 128],
                                rhs=csb[:, c, :, h * 512:(h + 1) * 512],
                                start=(c == 0),
                                stop=(c == CH - 1),
                                perf_mode=mybir.MatmulPerfMode.DoubleRow,
                            )
                    # PSUM fp32 -> SBUF fp16, one half per engine; h-outer MM
                    # order lets the ScalarE half start while half 1 computes
                    nc.scalar.copy(out=ssc[:, j, 0:512], in_=psum[:, 0:512])
                    nc.vector.tensor_copy(out=ssc[:, j, 512:1024], in_=psum[:, 512:1024])
                    if ot == NOT - 1:
                        # final tile: per-subtile stores on the (now idle) Sync
                        # ring so the tail doesn't wait on the SWDGE backlog
                        nc.sync.dma_start(out=sc_ext[ot, :, j, :], in_=ssc[:, j, :])
                # score export on the GpSimd SWDGE ring, two tiles per DMA:
                # keeps the Sync HWDGE ring free for x prefetch and halves the
                # SWDGE queue overhead (and its end-of-kernel drain backlog)
                if ot % 2 == 1 and ot < NOT - 1:
                    nc.gpsimd.dma_start(
                        out=sc_ext[ot - 1:ot + 1].rearrange("o p j k -> p o j k"),
                        in_=ssc2[:],
                    )
                elif ot == NOT - 2:
                    nc.gpsimd.dma_start(out=sc_ext[ot], in_=ssc2[:, 0])

    nc.finalize()
    return nc


def _prep_core(args):
    x, c = args
    x8 = x[c * NSH:(c + 1) * NSH].astype(E4)
    # xt[ot, ki, ch, ko, j*128+q] = x8[512*ot + 4*q + j, ch*256 + ko*128 + ki]
    v = x8.reshape(NOT, 128, 4, CH, 2, 128)      # [ot, q, j, ch, ko, ki]
    v = v.transpose(0, 5, 3, 4, 2, 1)            # [ot, ki, ch, ko, j, q]
    return np.ascontiguousarray(v).reshape(NOT, 128, CH, 2, MT)


def prepare_in_maps(x, C, Cnorm):
    x = np.ascontiguousarray(np.asarray(x, dtype=np.float32))
    C = np.ascontiguousarray(np.asarray(C, dtype=np.float32))

    from concurrent.futures import ThreadPoolExecutor
    with ThreadPoolExecutor(max_workers=8) as ex:
        xts = list(ex.map(_prep_core, [(x, c) for c in range(NCORES)]))

    # cb[ki, ch, ko, k] = C[ch*256 + ko*128 + ki, k]
    cb = np.ascontiguousarray(C.astype(E4).reshape(CH, 2, 128, K).transpose(2, 0, 1, 3))
    return [{"xt": xts[c], "cb": cb} for c in range(NCORES)]


def postprocess(results, x, C, Cnorm):
    """Capture candidates from fp8 scores, exactly re-score, gather."""
    x = np.asarray(x, dtype=np.float32)
    C = np.asarray(C, dtype=np.float32)
    bias = np.asarray(Cnorm, dtype=np.float32).reshape(K) * 0.5
    CT = np.ascontiguousarray(C.T)

    all_rows = []
    all_ks = []
    for c in range(NCORES):
        # sc[ot, p, j, k] holds the fp8 raw score of shard row 512*ot + 4*p + j
        sf = results[c]["sc"].reshape(NSH, K).astype(np.float32)
        sf -= bias
        m1 = sf.max(axis=1)
        rows_c, ks_c = np.nonzero(sf > (m1 - CAP_THETA)[:, None])
        all_rows.append(rows_c.astype(np.int64) + c * NSH)
        all_ks.append(ks_c.astype(np.int32))
    rows = np.concatenate(all_rows)     # sorted ascending by construction
    ks = np.concatenate(all_ks)

    # exact fp32 re-score of every candidate pair, grouped by codeword so
    # each group is one BLAS matvec (avoids a 1.5 GB codebook gather)
    order = np.argsort(ks, kind="stable")
    rs = rows[order]
    kss = ks[order]
    s_srt = np.empty(len(rows), dtype=np.float32)
    bounds = np.searchsorted(kss, np.arange(K + 1))
    for k in range(K):
        a, b = bounds[k], bounds[k + 1]
        if a != b:
            s_srt[a:b] = x[rs[a:b]] @ CT[k] - bias[k]
    s_ex = np.empty(len(rows), dtype=np.float32)
    s_ex[order] = s_srt

    # segmented argmax over candidate pairs (rows are sorted ascending)
    starts = np.flatnonzero(np.r_[True, np.diff(rows) != 0])
    counts = np.diff(np.r_[starts, len(rows)])
    seg_rows = rows[starts]
    seg_max = np.maximum.reduceat(s_ex, starts)
    win = s_ex == seg_max.repeat(counts)
    idx = np.empty(N, dtype=np.int64)
    idx[rows[win]] = ks[win]           # every row has >= 1 candidate (its own max)

    # float64 full-row recheck where the exact top-2 margin is inside fp32
    # noise (or the fp32 re-score produced an exact tie)
    s2 = s_ex.copy()
    s2[win] = -np.inf
    seg_second = np.maximum.reduceat(s2, starts)
    nwin = np.add.reduceat(win.astype(np.int64), starts)
    margin = np.where(nwin > 1, 0.0, seg_max - seg_second)
    tie_rows = seg_rows[margin < FINE_THETA]
    if tie_rows.size:
        xr = x[tie_rows].astype(np.float64)
        Cnorm64 = np.asarray(Cnorm, dtype=np.float64).reshape(1, K)
        dist = (
            np.sum(xr * xr, axis=1, keepdims=True)
            - 2.0 * (xr @ C.astype(np.float64))
            + Cnorm64
        )
        idx[tie_rows] = np.argmin(dist, axis=1)

    return CT[idx]


def kernel(x, C, Cnorm):
    in_maps = prepare_in_maps(x, C, Cnorm)
    nc = build_kernel()
    res = run_bass_kernel_spmd(nc, in_maps, core_ids=list(range(NCORES))).results
    return postprocess(res, x, C, Cnorm)


# revision 27
# speedup vs baseline: 1.0285x; 1.0158x over previous
"""VQ codebook kernel (nn_ApplyKmeans): dist = ||x||^2 - 2 x@C + Cnorm; argmin; gather.

Strategy (8 NeuronCores, data-parallel over rows of x):
  - Host: shard x by rows, cast to fp8 e4m3, and pre-tile into the DoubleRow
    SBUF layout: contraction dim d = c*256 + ko*128 + ki over CH=3 chunks,
    so each matmul contracts 256 dims (2 fp8 weights per PE cell).
  - Device per core (PE-bound design): raw scores = x.T @ C in fp8 e4m3 with
    perf_mode=DoubleRow (6 N=512 matmuls per 128-row subtile, ~216 ns each,
    ~2x the fp16 roofline), ScalarE+VectorE copy PSUM -> SBUF fp16 (one half
    each), DMA exports the raw score matrix (64 MB/core).
  - Host: subtract Cnorm/2 in fp32, take the fp8-score argmax, and exactly
    re-score every candidate within CAP_THETA of the per-row max (~4/row;
    fp8 noise absmax ~6 plus e4m3 export rounding ~5/score, so theta=26 captures
    the true argmax with enormous margin). A final float64 full-row re-score
    covers rows whose exact top-2 margin is below fp32 rescore noise.
"""

import sys

sys.path.insert(0, "/opt/trn_rl_repo")

import numpy as np
import ml_dtypes

import concourse.bass as bass
import concourse.mybir as mybir
from concourse import bacc
from concourse.tile import TileContext
from concourse.bass_utils import run_bass_kernel_spmd

N, D, K = 262144, 768, 1024
NCORES = 8
NSH = N // NCORES            # 32768 rows per core
CH = 3                       # DoubleRow contraction chunks of 256
MT = 512                     # rows per DMA tile
NOT = NSH // MT              # 64 outer tiles
CAP_THETA = 26.0             # fp8-score candidate-capture margin
FINE_THETA = 1e-3            # fp32-rescore tie margin -> float64 recheck

E4 = ml_dtypes.float8_e4m3


def build_kernel():
    nc = bacc.Bacc()
    xt_ext = nc.declare_dram_parameter("xt", [NOT, 128, CH, 2, MT], mybir.dt.float8e4, isOutput=False)
    cb_ext = nc.declare_dram_parameter("cb", [128, CH, 2, K], mybir.dt.float8e4, isOutput=False)
    sc_ext = nc.declare_dram_parameter("sc", [NOT, 128, MT // 128, K], mybir.dt.float8e4, isOutput=True)

    with TileContext(nc) as tc:
        with (
            tc.tile_pool(name="const", bufs=1) as const_pool,
            tc.tile_pool(name="xp", bufs=3) as xpool,
            tc.tile_pool(name="scp", bufs=4) as scpool,
            tc.tile_pool(name="ps", bufs=4, space="PSUM") as pspool,
        ):
            # Codebook on the Sync HWDGE ring, first x tile per-chunk on the
            # ScalarE ring: both chunk-0s land ~2us after the preamble so the
            # matmul stream starts as early as possible.
            csb = const_pool.tile([128, CH, 2, K], mybir.dt.float8e4)
            for c in range(CH):
                nc.sync.dma_start(out=csb[:, c, :, :], in_=cb_ext[:, c, :, :])



            for ot in range(NOT):
                xtile = xpool.tile([128, CH, 2, MT], mybir.dt.float8e4, tag="xt")
                if ot == 0:
                    for c in range(CH):
                        nc.scalar.dma_start(out=xtile[:, c, :, :], in_=xt_ext[0, :, c, :, :])
                else:
                    nc.sync.dma_start(out=xtile[:], in_=xt_ext[ot])
                if ot % 2 == 0:
                    ssc2 = scpool.tile([128, 2, MT // 128, K], mybir.dt.float8e4, tag="sc")
                ssc = ssc2[:, ot % 2]
                for j in range(MT // 128):
                    psum = pspool.tile([128, K], mybir.dt.float32, space="PSUM", tag="ps")
                    for h in range(2):
                        for c in range(CH):
                            nc.tensor.matmul(
                                out=psum[:, h * 512:(h + 1) * 512],
                                lhsT=xtile[:, c, :, j * 128:(j + 1) * 128],
                                rhs=csb[:, c, :, h * 512:(h + 1) * 512],
                                start=(c == 0),
                                stop=(c == CH - 1),
                                perf_mode=mybir.MatmulPerfMode.DoubleRow,
                            )
                    # PSUM fp32 -> SBUF fp16, one half per engine; h-outer MM
                    # order lets the ScalarE half start while half 1 computes
                    nc.scalar.copy(out=ssc[:, j, 0:512], in_=psum[:, 0:512])
                    nc.vector.tensor_copy(out=ssc[:, j, 512:1024], in_=psum[:, 512:1024])
                    if ot == NOT - 1:
                        # final tile: per-subtile stores on the (now idle) Sync
                        # ring so the tail doesn't wait on the SWDGE backlog
                        nc.sync.dma_start(out=sc_ext[ot, :, j, :], in_=ssc[:, j, :])
                # score export on the GpSimd SWDGE ring, two tiles per DMA:
                # keeps the Sync HWDGE ring free for x prefetch and halves the
                # SWDGE queue overhead (and its end-of-kernel drain backlog)
                if ot % 2 == 1 and ot < NOT - 1:
                    nc.gpsimd.dma_start(
                        out=sc_ext[ot - 1:ot + 1].rearrange("o p j k -> p o j k"),
                        in_=ssc2[:],
                    )
                elif ot == NOT - 2:
                    nc.gpsimd.dma_start(out=sc_ext[ot], in_=ssc2[:, 0])

    nc.finalize()
    return nc


def _prep_core(args):
    x, c = args
    x8 = x[c * NSH:(c + 1) * NSH].astype(E4)
    # xt[ot, ki, ch, ko, j*128+q] = x8[512*ot + 4*q + j, ch*256 + ko*128 + ki]
    v = x8.reshape(NOT, 128, 4, CH, 2, 128)      # [ot, q, j, ch, ko, ki]
    v = v.transpose(0, 5, 3, 4, 2, 1)            # [ot, ki, ch, ko, j, q]
    return np.ascontiguousarray(v).reshape(NOT, 128, CH, 2, MT)


def prepare_in_maps(x, C, Cnorm):
    x = np.ascontiguousarray(np.asarray(x, dtype=np.float32))
    C = np.ascontiguousarray(np.asarray(C, dtype=np.float32))

    from concurrent.futures import ThreadPoolExecutor
    with ThreadPoolExecutor(max_workers=8) as ex:
        xts = list(ex.map(_prep_core, [(x, c) for c in range(NCORES)]))

    # cb[ki, ch, ko, k] = C[ch*256 + ko*128 + ki, k]
    cb = np.ascontiguousarray(C.astype(E4).reshape(CH, 2, 128, K).transpose(2, 0, 1, 3))
    return [{"xt": xts[c], "cb": cb} for c in range(NCORES)]


def postprocess(results, x, C, Cnorm):
    """Capture candidates from fp8 scores, exactly re-score, gather."""
    x = np.asarray(x, dtype=np.float32)
    C = np.asarray(C, dtype=np.float32)
    bias = np.asarray(Cnorm, dtype=np.float32).reshape(K) * 0.5
    CT = np.ascontiguousarray(C.T)

    all_rows = []
    all_ks = []
    for c in range(NCORES):
        # sc[ot, p, j, k] holds the fp8 raw score of shard row 512*ot + 4*p + j
        sf = results[c]["sc"].reshape(NSH, K).astype(np.float32)
        sf -= bias
        m1 = sf.max(axis=1)
        rows_c, ks_c = np.nonzero(sf > (m1 - CAP_THETA)[:, None])
        all_rows.append(rows_c.astype(np.int64) + c * NSH)
        all_ks.append(ks_c.astype(np.int32))
    rows = np.concatenate(all_rows)     # sorted ascending by construction
    ks = np.concatenate(all_ks)

    # exact fp32 re-score of every candidate pair, grouped by codeword so
    # each group is one BLAS matvec (avoids a 1.5 GB codebook gather)
    order = np.argsort(ks, kind="stable")
    rs = rows[order]
    kss = ks[order]
    s_srt = np.empty(len(rows), dtype=np.float32)
    bounds = np.searchsorted(kss, np.arange(K + 1))
    for k in range(K):
        a, b = bounds[k], bounds[k + 1]
        if a != b:
            s_srt[a:b] = x[rs[a:b]] @ CT[k] - bias[k]
    s_ex = np.empty(len(rows), dtype=np.float32)
    s_ex[order] = s_srt

    # segmented argmax over candidate pairs (rows are sorted ascending)
    starts = np.flatnonzero(np.r_[True, np.diff(rows) != 0])
    counts = np.diff(np.r_[starts, len(rows)])
    seg_rows = rows[starts]
    seg_max = np.maximum.reduceat(s_ex, starts)
    win = s_ex == seg_max.repeat(counts)
    idx = np.empty(N, dtype=np.int64)
    idx[rows[win]] = ks[win]           # every row has >= 1 candidate (its own max)

    # float64 full-row recheck where the exact top-2 margin is inside fp32
    # noise (or the fp32 re-score produced an exact tie)
    s2 = s_ex.copy()
    s2[win] = -np.inf
    seg_second = np.maximum.reduceat(s2, starts)
    nwin = np.add.reduceat(win.astype(np.int64), starts)
    margin = np.where(nwin > 1, 0.0, seg_max - seg_second)
    tie_rows = seg_rows[margin < FINE_THETA]
    if tie_rows.size:
        xr = x[tie_rows].astype(np.float64)
        Cnorm64 = np.asarray(Cnorm, dtype=np.float64).reshape(1, K)
        dist = (
            np.sum(xr * xr, axis=1, keepdims=True)
            - 2.0 * (xr @ C.astype(np.float64))
            + Cnorm64
        )
        idx[tie_rows] = np.argmin(dist, axis=1)

    return CT[idx]


def kernel(x, C, Cnorm):
    in_maps = prepare_in_maps(x, C, Cnorm)
    nc = build_kernel()
    res = run_bass_kernel_spmd(nc, in_maps, core_ids=list(range(NCORES))).results
    return postprocess(res, x, C, Cnorm)
